# revision 37
# baseline (speedup 1.0000x reference)
"""DCGNN forward kernel for 8 Trainium2 NeuronCores.

The reference network is linear in x (the adjacency is built only from
coord), and the final output is just [B, 2].  The entire pipeline
  x -> Chebyshev(L) -> cheb_W -> (+cheb_b) -> 1x1 conv affine -> FC
therefore collapses to a single affine map

    out[b, n] = sum_k x_flat[b, k] * G[k, n] + const[n],

with G = [C*F_IN, NCLS] = [31744, 2] precomputed on the host from the
tiny parameter tensors.  The device kernel is a pure memory-bound
streaming reduction, so HBM bytes are the whole cost.

v3: x is streamed as fp8 (TRN FP8_EXP3 = e3m4, 4-bit mantissa; output
error 1.3e-2 measured on the real inputs vs the 2e-2 tolerance), which
quarters the HBM bytes vs f32: 8.1 MB per core.  x is pre-scaled by 2
on the host (max |2x| ~ 10.8 < 15.5 keeps subnormals away); the
inverse scale is folded into G, which rides as fp16 (the BIR verifier
forbids mixing fp32r with fp8 matmul operands; fp16's error
contribution is unmeasurable).

The DMA engines alone stream the 8 chunk reads at ~421 GB/s
(19.3 us/pass, probed), but concurrent PE SBUF reads slow them down,
and the loss is PORT-SPECIFIC (probed with dependency-free kernels):
8.1 MB of Fast-Weight-Load reads -> 324 GB/s, half that -> 365 GB/s,
while reads on the PE's OTHER SBUF port (the moving operand) cost
about half as much bandwidth per byte (4 MB moving -> 396 GB/s).  PE
speed is the opposing constraint: a k-tile on the weight path costs
2 matmuls x ~27 ns (FWL-bound cadence, measured back-to-back), on the
moving path 256 cycles (fp8 moving runs at bf16 speed, 1 col/cycle).
So the k-tiles are split 14 weight / 17 moving per 31-k-tile chunk
(blocked, measured optimum of the split scan):

  weight path: acc_w[b_tile 128, 2] += xT[128 k, 128 b].T @ G[128 k, 2]
               (x stationary via FWL, 2 matmuls per k-tile)
  moving path: acc_m[2, 256 b]     += G[128 k, 2].T @ xT[128 k, 256 b]
               (tiny G stationary, x moving, 1 matmul per k-tile)

which balances PE time (~20 us warm) against the recovered DMA rate,
and the two partial sums are added on the host
(out[b,n] = acc_w[b,n] + acc_m[n,b] + const[n]).

Per-core device pipeline (data-parallel over batch, no collectives):
  - the host pre-transposes each core's shard into k-major chunk blocks
    (chunk c is a contiguous [128, 31*256] fp8 block: col j*256+b of
    partition p holds x_q[b, (kt0_c+j)*128+p]), so every chunk DMA is
    one fully linear ~1 MB read and NO on-device transpose is needed
  - 8 chunks of 31 k-tiles alternate across BOTH HWDGE queues (SP and
    ACT), 6-deep buffered: one queue's ~600 ns per-DMA descriptor
    processing starves the 16 DMA engines between chunks
  - PSUM: two [128, 16] f32 weight-path accumulators (cols 2..15 are
    zero padding, see _GW) + one [2, 256] moving-path accumulator
  - output: both partials DMA'd out via the (otherwise idle) GPSIMD
    queue so their end-of-pass waits never block the chunk streams
  - measured steady state: ~23.5 us/pass, a jointly saturated pipeline
    (trace: DMA union busy 100% of wall at ~343 GB/s, tensor ~98%
    busy) — the contention-model optimum for this port split
"""

import numpy as np

_B, _C, _F_IN, _NCLS = 2048, 62, 512, 2
_THRESH = 0.1
_NCORES = 8
_B_LOC = _B // _NCORES            # 256
_KDIM = _C * _F_IN                # 31744
_P = 128
_KT = _KDIM // _P                 # 248 k-tiles
_NCHUNK = 8
_CKT = _KT // _NCHUNK             # 31 k-tiles per chunk
_XN = _KT * _P * _B_LOC           # total x elements per core
_XSCALE = 2.0                     # host pre-scale before fp8 round
_GW = 2                           # weight-path moving-operand width =
                                  # NCLS exactly: padding it wider streams
                                  # junk G columns through the moving port
                                  # (~0.9 MB/pass at width 16), which costs
                                  # measurable DMA bandwidth via port
                                  # contention


def _is_weight_kt(kt):
    # First 14 k-tiles of each chunk -> weight path, last 17 -> moving
    # path (scan: 12->23.9us, 14->23.4, 15->24.0, 16->23.7).  Blocked,
    # not interleaved: switching the stationary operand mode costs an
    # array drain (~128 cycles), so a j-parity interleave collapses the
    # matmul cadence (measured 55 us/pass vs 25).
    return (kt % _CKT) < 14


_W_KTS = [kt for kt in range(_KT) if _is_weight_kt(kt)]
_M_KTS = [kt for kt in range(_KT) if not _is_weight_kt(kt)]


def _precompute_g(coord, adj_w1, adj_b1, adj_w2, adj_b2, cheb_W, cheb_b,
                  conv_w, conv_b, fc_w, fc_b):
    """Fold every parameter into G [KDIM, NCLS] and const [NCLS].

    The adjacency MLP + threshold is done in f32 to mirror the reference
    bit-for-bit (the > 0.1 threshold must see the same values); the
    Laplacian / Chebyshev / folding run in f64 for accuracy.
    """
    f32 = np.float32
    coord = coord.astype(f32)
    h = np.maximum(coord @ adj_w1.astype(f32) + adj_b1.astype(f32), f32(0))
    w_star = (h @ adj_w2.astype(f32) + adj_b2.astype(f32))[..., 0]   # [C, C]

    C = w_star.shape[0]
    wd = w_star.astype(np.float64)
    eye = np.eye(C, dtype=bool)
    A = np.where((wd > _THRESH) & ~eye, wd, 0.0)
    deg = A.sum(axis=1)
    dis = np.where(deg > 0, 1.0 / np.sqrt(np.where(deg > 0, deg, 1.0)), 0.0)
    L = -(dis[:, None] * A * dis[None, :])

    K = cheb_W.shape[0]
    T = np.zeros((K, C, C))
    T[0] = np.eye(C)
    T[1] = L
    for k in range(2, K):
        T[k] = 2.0 * (L @ T[k - 1]) - T[k - 2]

    ncls = fc_w.shape[1]
    Fc = fc_w.astype(np.float64).reshape(C, -1, ncls)               # [C, F_OUT, N]
    cw = float(np.asarray(conv_w).reshape(-1)[0])
    cb = float(np.asarray(conv_b).reshape(-1)[0])

    G = np.zeros((C, cheb_W.shape[1], ncls))
    for k in range(K):
        U = np.einsum('if,cfn->icn', cheb_W[k].astype(np.float64), Fc,
                      optimize=True)
        G += np.einsum('cj,icn->jin', T[k], U, optimize=True)
    G *= cw

    const = ((cw * np.tile(cheb_b.astype(np.float64), C) + cb)
             @ fc_w.astype(np.float64)) + fc_b.astype(np.float64)
    return G.reshape(C * cheb_W.shape[1], ncls).astype(f32), const.astype(f32)


_NC_CACHE = {}


def _build_nc(reps=1):
    """Build the bass module. reps>1 unrolls back-to-back passes —
    used only for steady-state throughput timing (see perf.py)."""
    if reps in _NC_CACHE:
        return _NC_CACHE[reps]

    import concourse.mybir as mybir
    import concourse.tile as tile
    from concourse import bacc

    f32 = mybir.dt.float32
    f16 = mybir.dt.float16
    f8 = mybir.dt.float8e3

    # Bacc (not plain Bass): its finalize() runs the TRN2 sync-wait
    # legalization that walrus codegen requires.
    nc = bacc.Bacc()
    x_dram = nc.declare_dram_parameter("x_shard", [_XN], f8, isOutput=False)
    g_dram = nc.declare_dram_parameter("g", [_P, _KT * _GW], f16,
                                       isOutput=False)
    outw_dram = nc.declare_dram_parameter("out_w", [_P, 2 * _NCLS], f32,
                                          isOutput=True)
    outm_dram = nc.declare_dram_parameter("out_m", [_NCLS, _B_LOC], f32,
                                          isOutput=True)

    with tile.TileContext(nc) as tc:
        with (
            tc.tile_pool(name="const", bufs=1) as const_pool,
            tc.tile_pool(name="x", bufs=6) as x_pool,
            tc.tile_pool(name="out", bufs=2) as out_pool,
            tc.tile_pool(name="acc", bufs=2, space="PSUM") as acc_pool,
        ):
            g_r = const_pool.tile([_P, _KT * _GW], f16, tag="g")
            nc.sync.dma_start(out=g_r[:], in_=g_dram[:])

            def one_pass():
                acc_w = [acc_pool.tile([_P, _GW], f32, tag=f"accw{bt}",
                                       name=f"accw{bt}")
                         for bt in range(2)]
                acc_m = acc_pool.tile([_NCLS, _B_LOC], f32, tag="accm",
                                      name="acc_m")
                for c in range(_NCHUNK):
                    ncols = _CKT * _B_LOC
                    xt = x_pool.tile([_P, ncols], f8, tag="x")
                    off = c * _P * ncols
                    # Alternate chunks across both HWDGE queues (SP and
                    # ACT): a single queue's ~600ns per-DMA descriptor
                    # processing starves the 16 DMA engines between
                    # chunks.  (64-partition splits are much worse —
                    # they break the line-to-engine striping.)
                    eng = nc.sync if c % 2 == 0 else nc.scalar
                    eng.dma_start(
                        out=xt[:],
                        in_=x_dram[off:off + _P * ncols]
                            .rearrange("(p f) -> p f", p=_P))
                    for j in range(_CKT):
                        kt = c * _CKT + j
                        if _is_weight_kt(kt):
                            for bt in range(2):
                                nc.tensor.matmul(
                                    acc_w[bt][:],
                                    xt[:, j * _B_LOC + bt * _P:
                                          j * _B_LOC + (bt + 1) * _P],
                                    g_r[:, kt * _GW:(kt + 1) * _GW],
                                    start=(kt == _W_KTS[0]),
                                    stop=(kt == _W_KTS[-1]))
                        else:
                            nc.tensor.matmul(
                                acc_m[:],
                                g_r[:, kt * _GW:kt * _GW + _NCLS],
                                xt[:, j * _B_LOC:(j + 1) * _B_LOC],
                                start=(kt == _M_KTS[0]),
                                stop=(kt == _M_KTS[-1]))

                outw_sb = out_pool.tile([_P, 2 * _NCLS], f32, tag="outw")
                for bt in range(2):
                    nc.vector.tensor_copy(
                        outw_sb[:, bt * _NCLS:(bt + 1) * _NCLS],
                        acc_w[bt][:, 0:_NCLS])
                outm_sb = out_pool.tile([_NCLS, _B_LOC], f32, tag="outm")
                nc.vector.tensor_copy(outm_sb[:], acc_m[:])
                # Out DMAs ride the (otherwise idle) GPSIMD queue: on
                # SP/ACT their end-of-pass semaphore wait would head-of-
                # line block the chunk DMA stream for the next pass.
                nc.gpsimd.dma_start(out=outw_dram[:], in_=outw_sb[:])
                nc.gpsimd.dma_start(out=outm_dram[:], in_=outm_sb[:])

            # reps>1 is the timing build: unrolled python loop (NOT
            # tc.For_i) so consecutive passes pipeline — For_i inserts a
            # ~3 us all-engine semaphore-reset barrier per iteration,
            # which is loop mechanics, not kernel cost.  Double-buffered
            # PSUM accumulators + out tiles let pass k+1's matmuls start
            # while pass k's result drains.
            for _ in range(reps):
                one_pass()

    nc.finalize()

    _NC_CACHE[reps] = nc
    return nc


def _make_in_maps(x, g_flat):
    import ml_dtypes

    # Fold the host-side x pre-scale into G.
    # Device layout: g_host[p, t*_GW + n] = G[t*128 + p, n] / XSCALE
    g3 = (g_flat / _XSCALE).reshape(_KT, _P, _NCLS).transpose(1, 0, 2)
    g_host = np.zeros((_P, _KT, _GW), np.float16)
    g_host[:, :, :_NCLS] = g3
    g_host = np.ascontiguousarray(g_host.reshape(_P, -1))

    # fp8 e3m4 round of the whole batch at once (RNE via ml_dtypes)
    x_flat = np.asarray(x, dtype=np.float32).reshape(_B, _KDIM)
    xq = (x_flat * np.float32(_XSCALE)).astype(ml_dtypes.float8_e3m4)

    in_maps = []
    for i in range(_NCORES):
        x_loc = xq[i * _B_LOC:(i + 1) * _B_LOC]
        # Chunk-block layout: chunk c tile [p, j*256+b] = x_q[b, kt*128+p]
        # with kt = c*31+j, so each chunk DMA is one linear ~1 MB read
        # and both per-matmul slice shapes ([128 k, 128 b] stationary,
        # [128 k, 256 b] moving) have k on partitions — no on-device
        # transpose anywhere.
        xh = (x_loc.reshape(_B_LOC, _NCHUNK, _CKT, _P)   # [b, c, j, p]
              .transpose(1, 3, 2, 0)                      # [c, p, j, b]
              .reshape(-1))
        in_maps.append({"x_shard": np.ascontiguousarray(xh), "g": g_host})
    return in_maps


def kernel(x, coord, adj_w1, adj_b1, adj_w2, adj_b2, cheb_W, cheb_b,
           conv_w, conv_b, fc_w, fc_b):
    from concourse.bass_utils import run_bass_kernel_spmd

    g_flat, const = _precompute_g(coord, adj_w1, adj_b1, adj_w2, adj_b2,
                                  cheb_W, cheb_b, conv_w, conv_b, fc_w, fc_b)
    in_maps = _make_in_maps(x, g_flat)

    nc = _build_nc()
    res = run_bass_kernel_spmd(nc, in_maps, core_ids=list(range(_NCORES)))
    global _LAST_RESULTS
    _LAST_RESULTS = res

    # out_w[p, bt*2+n] = weight-path partial of out[bt*128+p, n];
    # out_m[n, b] = moving-path partial.  Sum them per core.
    outs = []
    for r in res.results:
        ow = (r["out_w"].reshape(_P, 2, _NCLS).transpose(1, 0, 2)
              .reshape(_B_LOC, _NCLS))
        outs.append(ow + r["out_m"].T)
    out = np.concatenate(outs, axis=0)
    return (out + const[None, :]).astype(np.float32)


_LAST_RESULTS = None


# revision 38
# speedup vs baseline: 1.0079x; 1.0079x over previous
"""DCGNN forward kernel for 8 Trainium2 NeuronCores.

The reference network is linear in x (the adjacency is built only from
coord), and the final output is just [B, 2].  The entire pipeline
  x -> Chebyshev(L) -> cheb_W -> (+cheb_b) -> 1x1 conv affine -> FC
therefore collapses to a single affine map

    out[b, n] = sum_k x_flat[b, k] * G[k, n] + const[n],

with G = [C*F_IN, NCLS] = [31744, 2] precomputed on the host from the
tiny parameter tensors.  The device kernel is a pure memory-bound
streaming reduction, so HBM bytes are the whole cost.

v3: x is streamed as fp8 (TRN FP8_EXP3 = e3m4, 4-bit mantissa; output
error 1.3e-2 measured on the real inputs vs the 2e-2 tolerance), which
quarters the HBM bytes vs f32: 8.1 MB per core.  x is pre-scaled by 2
on the host (max |2x| ~ 10.8 < 15.5 keeps subnormals away); the
inverse scale is folded into G, which rides as fp16 (the BIR verifier
forbids mixing fp32r with fp8 matmul operands; fp16's error
contribution is unmeasurable).

The DMA engines alone stream the 8 chunk reads at ~421 GB/s
(19.3 us/pass, probed), but concurrent PE SBUF reads slow them down,
and the loss is PORT-SPECIFIC (probed with dependency-free kernels):
8.1 MB of Fast-Weight-Load reads -> 324 GB/s, half that -> 365 GB/s,
while reads on the PE's OTHER SBUF port (the moving operand) cost
about half as much bandwidth per byte (4 MB moving -> 396 GB/s).  PE
speed is the opposing constraint: a k-tile on the weight path costs
2 matmuls x ~27 ns (FWL-bound cadence, measured back-to-back), on the
moving path 256 cycles (fp8 moving runs at bf16 speed, 1 col/cycle).
So the k-tiles are split 14 weight / 17 moving per 31-k-tile chunk
(blocked, measured optimum of the split scan):

  weight path: acc_w[b_tile 128, 2] += xT[128 k, 128 b].T @ G[128 k, 2]
               (x stationary via FWL, 2 matmuls per k-tile)
  moving path: acc_m[2, 256 b]     += G[128 k, 2].T @ xT[128 k, 256 b]
               (tiny G stationary, x moving, 1 matmul per k-tile)

which balances PE time (~20 us warm) against the recovered DMA rate,
and the two partial sums are added on the host
(out[b,n] = acc_w[b,n] + acc_m[n,b] + const[n]).

Per-core device pipeline (data-parallel over batch, no collectives):
  - the host pre-transposes each core's shard into k-major chunk blocks
    (chunk c is a contiguous [128, 31*256] fp8 block: col j*256+b of
    partition p holds x_q[b, (kt0_c+j)*128+p]), so every chunk DMA is
    one fully linear ~1 MB read and NO on-device transpose is needed
  - 8 chunks of 31 k-tiles alternate across BOTH HWDGE queues (SP and
    ACT), 6-deep buffered: one queue's ~600 ns per-DMA descriptor
    processing starves the 16 DMA engines between chunks
  - PSUM: two [128, 2] f32 weight-path accumulators (one per b-tile)
    + one [2, 256] moving-path accumulator
  - output: both partials DMA'd out via the (otherwise idle) GPSIMD
    queue so their end-of-pass waits never block the chunk streams
  - measured steady state: ~23.5 us/pass, a jointly saturated pipeline
    (trace: DMA union busy 100% of wall at ~343 GB/s, tensor ~98%
    busy) — the contention-model optimum for this port split
"""

import numpy as np

_B, _C, _F_IN, _NCLS = 2048, 62, 512, 2
_THRESH = 0.1
_NCORES = 8
_B_LOC = _B // _NCORES            # 256
_KDIM = _C * _F_IN                # 31744
_P = 128
_KT = _KDIM // _P                 # 248 k-tiles
_NCHUNK = 8
_CKT = _KT // _NCHUNK             # 31 k-tiles per chunk
_XN = _KT * _P * _B_LOC           # total x elements per core
_XSCALE = 2.0                     # host pre-scale before fp8 round
_GW = 2                           # weight-path moving-operand width =
                                  # NCLS exactly: padding it wider streams
                                  # junk G columns through the moving port
                                  # (~0.9 MB/pass at width 16), which costs
                                  # measurable DMA bandwidth via port
                                  # contention


def _is_weight_kt(kt):
    # First 14 k-tiles of each chunk -> weight path, last 17 -> moving
    # path (scan: 12->23.9us, 14->23.4, 15->24.0, 16->23.7).  Blocked,
    # not interleaved: switching the stationary operand mode costs an
    # array drain (~128 cycles), so a j-parity interleave collapses the
    # matmul cadence (measured 55 us/pass vs 25).
    return (kt % _CKT) < 14


_W_KTS = [kt for kt in range(_KT) if _is_weight_kt(kt)]
_M_KTS = [kt for kt in range(_KT) if not _is_weight_kt(kt)]


def _precompute_g(coord, adj_w1, adj_b1, adj_w2, adj_b2, cheb_W, cheb_b,
                  conv_w, conv_b, fc_w, fc_b):
    """Fold every parameter into G [KDIM, NCLS] and const [NCLS].

    The adjacency MLP + threshold is done in f32 to mirror the reference
    bit-for-bit (the > 0.1 threshold must see the same values); the
    Laplacian / Chebyshev / folding run in f64 for accuracy.
    """
    f32 = np.float32
    coord = coord.astype(f32)
    h = np.maximum(coord @ adj_w1.astype(f32) + adj_b1.astype(f32), f32(0))
    w_star = (h @ adj_w2.astype(f32) + adj_b2.astype(f32))[..., 0]   # [C, C]

    C = w_star.shape[0]
    wd = w_star.astype(np.float64)
    eye = np.eye(C, dtype=bool)
    A = np.where((wd > _THRESH) & ~eye, wd, 0.0)
    deg = A.sum(axis=1)
    dis = np.where(deg > 0, 1.0 / np.sqrt(np.where(deg > 0, deg, 1.0)), 0.0)
    L = -(dis[:, None] * A * dis[None, :])

    K = cheb_W.shape[0]
    T = np.zeros((K, C, C))
    T[0] = np.eye(C)
    T[1] = L
    for k in range(2, K):
        T[k] = 2.0 * (L @ T[k - 1]) - T[k - 2]

    ncls = fc_w.shape[1]
    Fc = fc_w.astype(np.float64).reshape(C, -1, ncls)               # [C, F_OUT, N]
    cw = float(np.asarray(conv_w).reshape(-1)[0])
    cb = float(np.asarray(conv_b).reshape(-1)[0])

    G = np.zeros((C, cheb_W.shape[1], ncls))
    for k in range(K):
        U = np.einsum('if,cfn->icn', cheb_W[k].astype(np.float64), Fc,
                      optimize=True)
        G += np.einsum('cj,icn->jin', T[k], U, optimize=True)
    G *= cw

    const = ((cw * np.tile(cheb_b.astype(np.float64), C) + cb)
             @ fc_w.astype(np.float64)) + fc_b.astype(np.float64)
    return G.reshape(C * cheb_W.shape[1], ncls).astype(f32), const.astype(f32)


_NC_CACHE = {}


def _build_nc(reps=1):
    """Build the bass module. reps>1 unrolls back-to-back passes —
    used only for steady-state throughput timing (see perf.py)."""
    if reps in _NC_CACHE:
        return _NC_CACHE[reps]

    import concourse.mybir as mybir
    import concourse.tile as tile
    from concourse import bacc

    f32 = mybir.dt.float32
    f16 = mybir.dt.float16
    f8 = mybir.dt.float8e3

    # Bacc (not plain Bass): its finalize() runs the TRN2 sync-wait
    # legalization that walrus codegen requires.
    nc = bacc.Bacc()
    x_dram = nc.declare_dram_parameter("x_shard", [_XN], f8, isOutput=False)
    g_dram = nc.declare_dram_parameter("g", [_P, _KT * _GW], f16,
                                       isOutput=False)
    outw_dram = nc.declare_dram_parameter("out_w", [_P, 2 * _NCLS], f32,
                                          isOutput=True)
    outm_dram = nc.declare_dram_parameter("out_m", [_NCLS, _B_LOC], f32,
                                          isOutput=True)

    with tile.TileContext(nc) as tc:
        with (
            tc.tile_pool(name="const", bufs=1) as const_pool,
            tc.tile_pool(name="x", bufs=6) as x_pool,
            tc.tile_pool(name="out", bufs=2) as out_pool,
            tc.tile_pool(name="acc", bufs=2, space="PSUM") as acc_pool,
        ):
            g_r = const_pool.tile([_P, _KT * _GW], f16, tag="g")
            nc.sync.dma_start(out=g_r[:], in_=g_dram[:])

            def one_pass():
                acc_w = [acc_pool.tile([_P, _GW], f32, tag=f"accw{bt}",
                                       name=f"accw{bt}")
                         for bt in range(2)]
                acc_m = acc_pool.tile([_NCLS, _B_LOC], f32, tag="accm",
                                      name="acc_m")
                for c in range(_NCHUNK):
                    ncols = _CKT * _B_LOC
                    xt = x_pool.tile([_P, ncols], f8, tag="x")
                    off = c * _P * ncols
                    # Alternate chunks across both HWDGE queues (SP and
                    # ACT): a single queue's ~600ns per-DMA descriptor
                    # processing starves the 16 DMA engines between
                    # chunks.  (64-partition splits are much worse —
                    # they break the line-to-engine striping.)
                    eng = nc.sync if c % 2 == 0 else nc.scalar
                    eng.dma_start(
                        out=xt[:],
                        in_=x_dram[off:off + _P * ncols]
                            .rearrange("(p f) -> p f", p=_P))
                    for j in range(_CKT):
                        kt = c * _CKT + j
                        if _is_weight_kt(kt):
                            for bt in range(2):
                                nc.tensor.matmul(
                                    acc_w[bt][:],
                                    xt[:, j * _B_LOC + bt * _P:
                                          j * _B_LOC + (bt + 1) * _P],
                                    g_r[:, kt * _GW:(kt + 1) * _GW],
                                    start=(kt == _W_KTS[0]),
                                    stop=(kt == _W_KTS[-1]))
                        else:
                            nc.tensor.matmul(
                                acc_m[:],
                                g_r[:, kt * _GW:kt * _GW + _NCLS],
                                xt[:, j * _B_LOC:(j + 1) * _B_LOC],
                                start=(kt == _M_KTS[0]),
                                stop=(kt == _M_KTS[-1]))

                outw_sb = out_pool.tile([_P, 2 * _NCLS], f32, tag="outw")
                for bt in range(2):
                    nc.vector.tensor_copy(
                        outw_sb[:, bt * _NCLS:(bt + 1) * _NCLS],
                        acc_w[bt][:, 0:_NCLS])
                outm_sb = out_pool.tile([_NCLS, _B_LOC], f32, tag="outm")
                nc.vector.tensor_copy(outm_sb[:], acc_m[:])
                # Out DMAs ride the (otherwise idle) GPSIMD queue: on
                # SP/ACT their end-of-pass semaphore wait would head-of-
                # line block the chunk DMA stream for the next pass.
                nc.gpsimd.dma_start(out=outw_dram[:], in_=outw_sb[:])
                nc.gpsimd.dma_start(out=outm_dram[:], in_=outm_sb[:])

            # reps>1 is the timing build: unrolled python loop (NOT
            # tc.For_i) so consecutive passes pipeline — For_i inserts a
            # ~3 us all-engine semaphore-reset barrier per iteration,
            # which is loop mechanics, not kernel cost.  Double-buffered
            # PSUM accumulators + out tiles let pass k+1's matmuls start
            # while pass k's result drains.
            for _ in range(reps):
                one_pass()

    nc.finalize()

    _NC_CACHE[reps] = nc
    return nc


def _make_in_maps(x, g_flat):
    import ml_dtypes

    # Fold the host-side x pre-scale into G.
    # Device layout: g_host[p, t*_GW + n] = G[t*128 + p, n] / XSCALE
    g3 = (g_flat / _XSCALE).reshape(_KT, _P, _NCLS).transpose(1, 0, 2)
    g_host = np.zeros((_P, _KT, _GW), np.float16)
    g_host[:, :, :_NCLS] = g3
    g_host = np.ascontiguousarray(g_host.reshape(_P, -1))

    # fp8 e3m4 round of the whole batch at once (RNE via ml_dtypes)
    x_flat = np.asarray(x, dtype=np.float32).reshape(_B, _KDIM)
    xq = (x_flat * np.float32(_XSCALE)).astype(ml_dtypes.float8_e3m4)

    in_maps = []
    for i in range(_NCORES):
        x_loc = xq[i * _B_LOC:(i + 1) * _B_LOC]
        # Chunk-block layout: chunk c tile [p, j*256+b] = x_q[b, kt*128+p]
        # with kt = c*31+j, so each chunk DMA is one linear ~1 MB read
        # and both per-matmul slice shapes ([128 k, 128 b] stationary,
        # [128 k, 256 b] moving) have k on partitions — no on-device
        # transpose anywhere.
        xh = (x_loc.reshape(_B_LOC, _NCHUNK, _CKT, _P)   # [b, c, j, p]
              .transpose(1, 3, 2, 0)                      # [c, p, j, b]
              .reshape(-1))
        in_maps.append({"x_shard": np.ascontiguousarray(xh), "g": g_host})
    return in_maps


def kernel(x, coord, adj_w1, adj_b1, adj_w2, adj_b2, cheb_W, cheb_b,
           conv_w, conv_b, fc_w, fc_b):
    from concourse.bass_utils import run_bass_kernel_spmd

    g_flat, const = _precompute_g(coord, adj_w1, adj_b1, adj_w2, adj_b2,
                                  cheb_W, cheb_b, conv_w, conv_b, fc_w, fc_b)
    in_maps = _make_in_maps(x, g_flat)

    nc = _build_nc()
    res = run_bass_kernel_spmd(nc, in_maps, core_ids=list(range(_NCORES)))
    global _LAST_RESULTS
    _LAST_RESULTS = res

    # out_w[p, bt*2+n] = weight-path partial of out[bt*128+p, n];
    # out_m[n, b] = moving-path partial.  Sum them per core.
    outs = []
    for r in res.results:
        ow = (r["out_w"].reshape(_P, 2, _NCLS).transpose(1, 0, 2)
              .reshape(_B_LOC, _NCLS))
        outs.append(ow + r["out_m"].T)
    out = np.concatenate(outs, axis=0)
    return (out + const[None, :]).astype(np.float32)


_LAST_RESULTS = None
